# revision 35
# baseline (speedup 1.0000x reference)
"""Trainium2 Bass kernel for nn_ComposeImgLoss (8-core data-parallel).

Contract: kernel(**inputs) takes the FULL inputs
    GT   (8, 4, 128, 128) f32
    Pred (8, 6, 14, 4, 128, 128) f32
and returns the FULL scalar loss (f32), matching reference.reference().

Strategy (1 sample per core, 8 cores). Measured op modes on this HW:
plain tensor_scalar / tensor_copy are 2x (fp32), tensor_tensor is 2x in
bf16 only; all accum/reduce/STT/ACT paths are 1x; DMA is HBM-limited at
~264 GB/s/core (~84us for the 21MB Pred slice); first two collectives
are cold (~33/24us), later ones ~10us.

  phase 0: Pred DMA'd in 4-slot chunks alternating across both HW DGE
    rings (sync=SP, scalar=Activation) so completion semaphores fire
    every ~4us; two dummy AllReduces fire at t=0 to warm the CC cores.
  phase 1 (under the DMA shadow): min/max per chunk via in-place
    tensor_scalar identity + accum_out (elementwise out == input, so no
    scratch and no data change); gen_L+gen_A+gen_W summed on PE via
    identity-matmul accumulation.
  collective 1: AllReduce(max) of [max, -min].
  phase 2: thresholds broadcast via ones-matmul; count indicators:
    DVE is_gt (0/1) for the alpha-gated pair (mn, t02) + ACT Sign (+-1)
    for t08/t04/t06; per-(slot,chan) sums via bf16 TT half-fold trees
    (exact: integer partial sums <= 128); partition fold via one bf16
    ones-matmul; type_list vals/membership on a [1,42] row; sigmoid on
    ACT; weighted slot-sum on PE via scaled-identity PSUM accumulation.
  collective 2: AllReduce(max) of region-sum [max, -min]; composite =
    clip(region_contrib + G3, 0, 1); SSE via fused square+accum and a
    ones-matmul partition fold.
  host: loss = sum(sse_core) / (8*3*128*128)
"""

import numpy as np

import concourse.bass as bass
import concourse.bacc as bacc
import concourse.tile as tile
from concourse import mybir
from concourse.masks import make_identity
from concourse.bass_utils import run_bass_kernel_spmd

import os
NO_COLL = os.environ.get("NO_COLL", "0") == "1"
NO_WARM = os.environ.get("NO_WARM", "0") == "1"

F32 = mybir.dt.float32
BF16 = mybir.dt.bfloat16
OP = mybir.AluOpType
AF = mybir.ActivationFunctionType
AX = mybir.AxisListType

N_CORES = 8
SQE, H, W = 14, 128, 128
SC = 4 * W            # cols per slot (4 channels x 128 w)
AT = SQE * SC         # 7168 cols per attri
RES = [0, 2, 3, 4, 5]  # SBUF-resident attris; attri 1 is streamed
RB = {a: i * AT for i, a in enumerate(RES)}
NPIX = N_CORES * 3 * H * W  # denominator of the global mean
NBLK = float(H * W)         # pixels per (slot, chan) block


def build():
    nc = bacc.Bacc("TRN2", target_bir_lowering=False, debug=False,
                   num_devices=N_CORES)
    pred = nc.dram_tensor("Pred", [6, SQE, 4, H, W], F32, kind="ExternalInput")
    gt = nc.dram_tensor("GT", [4, H, W], F32, kind="ExternalInput")
    sse = nc.dram_tensor("sse", [1, 1], F32, kind="ExternalOutput")
    dbg = nc.dram_tensor("dbg", [1, 16], F32, kind="ExternalOutput")
    dbg2 = nc.dram_tensor("dbg2", [1, 256], F32, kind="ExternalOutput")

    with tile.TileContext(nc) as tc:
        with (
            tc.tile_pool(name="big", bufs=1) as big,
            tc.tile_pool(name="a1s", bufs=7) as a1s,
            tc.tile_pool(name="mid", bufs=1) as mid,
            tc.tile_pool(name="small", bufs=1) as small,
            tc.tile_pool(name="psum", bufs=1, space="PSUM") as psum,
            tc.tile_pool(name="dram", bufs=1, space="DRAM") as dram,
        ):
            # ---------------- tiles ----------------
            r5 = big.tile([128, 5 * AT], F32)          # 140KB/part
            ib0 = mid.tile([128, SQE * 3 * W], BF16)   # indicator bufs
            ib1 = mid.tile([128, SQE * 3 * W], BF16)
            alm = mid.tile([128, SQE * W], BF16)       # alpha mask -> region
            law = mid.tile([128, 3 * W], F32)          # raw L+A+W -> G3 -> SSE
            acc = mid.tile([128, 3 * W], BF16)         # weighted slot-sum
            gtn = mid.tile([128, 3 * W], F32)

            ident = small.tile([128, 128], F32)
            ones_c = small.tile([128, 1], F32)
            ones_cb = small.tile([128, 1], BF16)
            ones_r = small.tile([1, 128], F32)
            mmax = small.tile([128, 32], F32)          # per-chunk max cols
            mmin = small.tile([128, 32], F32)          # per-chunk min cols
            mm2 = small.tile([128, 2], F32)
            mtmp = small.tile([128, 1], F32)
            cnt = small.tile([128, 224], BF16)         # folded count sums
            gsb = small.tile([1, 2], F32)
            rsb = small.tile([1, 2], F32)
            rowb = small.tile([1, 16], F32)
            rtmp = small.tile([1, 1], F32)
            dd = small.tile([1, 1], F32)
            invd = small.tile([1, 1], F32)
            rinvd = small.tile([1, 1], F32)
            rrow = small.tile([1, 2], F32)
            crow = small.tile([1, 224], F32)
            drow = small.tile([1, 126], F32)           # C2d C1d C0d
            rows = small.tile([1, 300], F32)
            srow = small.tile([1, 112], F32)
            thrb = small.tile([128, 12], F32)
            typb = small.tile([128, 42], F32)
            rmm2 = small.tile([128, 2], F32)
            rcp = small.tile([128, 2], F32)
            ssecol = small.tile([128, 1], F32)
            sse_sb = small.tile([1, 1], F32)
            warm = small.tile([128, 1], F32)
            warm2 = small.tile([128, 1], F32)
            zrow = small.tile([16, 1], F32)

            cin1 = dram.tile([16, 1], F32)
            cout1 = dram.tile([16, 1], F32, addr_space="Shared")
            cin2 = dram.tile([16, 1], F32)
            cout2 = dram.tile([16, 1], F32, addr_space="Shared")
            cinw = dram.tile([16, 1], F32)
            coutw = dram.tile([16, 1], F32, addr_space="Shared")
            cinw2 = dram.tile([16, 1], F32)
            coutw2 = dram.tile([16, 1], F32, addr_space="Shared")
            cinw3 = dram.tile([16, 1], F32)
            coutw3 = dram.tile([16, 1], F32, addr_space="Shared")
            gpad = small.tile([16, 1], F32)
            rpad = small.tile([16, 1], F32)

            # ---------------- warmup collectives ----------------
            nc.vector.memset(zrow[:], 0.0)
            nc.gpsimd.dma_start(out=cinw[:], in_=zrow[:])
            if not (NO_COLL or NO_WARM):
                nc.gpsimd.collective_compute(
                    "AllReduce", OP.max, replica_groups=[list(range(N_CORES))],
                    ins=[cinw.opt()], outs=[coutw.opt()])
                nc.gpsimd.dma_start(out=cinw2[:], in_=zrow[:])
                nc.gpsimd.collective_compute(
                    "AllReduce", OP.max, replica_groups=[list(range(N_CORES))],
                    ins=[cinw2.opt()], outs=[coutw2.opt()])
                nc.gpsimd.dma_start(out=cinw3[:], in_=zrow[:])
                nc.gpsimd.collective_compute(
                    "AllReduce", OP.max, replica_groups=[list(range(N_CORES))],
                    ins=[cinw3.opt()], outs=[coutw3.opt()])

            # ---------------- phase 0: DMA fan-out ----------------
            pr = pred.ap()

            def load(engine, a, s0, s1):
                off = RB[a] + s0 * SC
                engine.dma_start(
                    out=r5[:, off:off + (s1 - s0) * SC].rearrange(
                        "h (s c w) -> h s c w", s=s1 - s0, c=4),
                    in_=pr[a][s0:s1].rearrange("s c h w -> h s c w"))

            # 4-slot chunks alternating across the two rings so completion
            # semaphores fire every ~4us and DVE minmax tracks arrivals.
            CH = [(0, 4), (4, 8), (8, 12), (12, 14)]
            pa1 = pr[1].rearrange("s c h w -> h s c w")
            a1ch = []
            # a2/a3 in fine 4-slot chunks (DVE ramp-up); then a4/a5/a0
            # as halves split across BOTH rings (each ring sustains only
            # ~118GB/s when both run, so halves land ~2x sooner than
            # full-attri loads), with a1 chunks interleaved between them
            # so arrivals track DVE's minmax consumption.
            def load_a1(k, eng):
                ch = a1s.tile([128, 1024], F32, tag="a1chunk")
                eng.dma_start(
                    out=ch[:].rearrange("h (s c w) -> h s c w", s=2, c=4),
                    in_=pa1[:, 2 * k:2 * k + 2])
                a1ch.append(ch)

            # a2 head extra-fine so DVE starts by ~12us
            CH2 = [(0, 2), (2, 4), (4, 8), (8, 11), (11, 14)]
            for q, (s0, s1) in enumerate(CH2):
                load(nc.sync if q % 2 == 0 else nc.scalar, 2, s0, s1)
            for q, (s0, s1) in enumerate(CH):
                load(nc.scalar if q % 2 == 0 else nc.sync, 3, s0, s1)
            load(nc.sync, 4, 0, 7)
            load(nc.scalar, 4, 7, SQE)
            load_a1(0, nc.sync)
            load_a1(1, nc.scalar)
            load(nc.sync, 5, 0, 7)
            load(nc.scalar, 5, 7, SQE)
            load_a1(2, nc.sync)
            load_a1(3, nc.scalar)
            load_a1(4, nc.sync)
            load_a1(5, nc.scalar)
            load(nc.sync, 0, 0, 7)
            load_a1(6, nc.scalar)
            load(nc.scalar, 0, 7, SQE)
            nc.sync.dma_start(out=gtn[:].rearrange("h (c w) -> h c w", w=W),
                              in_=gt.ap()[0:3].rearrange("c h w -> h c w"))

            # ---------------- constants ----------------
            make_identity(nc, ident[:])
            nc.vector.memset(ones_c[:], 1.0)
            nc.vector.memset(ones_cb[:], 1.0)
            nc.vector.memset(ones_r[:], 1.0)
            # ACT table warmups: sigmoid then sign, so the Sign table is
            # resident when phase-2 counts start.
            nc.scalar.activation(warm[:], ones_c[:], AF.Sigmoid)
            nc.scalar.activation(warm2[:], ones_c[:], AF.Sign)

            # ---------------- phase 1: min/max + LAW ----------------
            # In-place tensor_scalar identity (x*1.0 == x, bit-exact) +
            # accum_out; no scratch tile, no value change. 1x mode, but
            # chunks track DMA arrivals so DVE streams behind the load.
            mmcol = [0]

            def mmts(src_ap):
                c = mmcol[0]
                nc.vector.tensor_scalar(
                    out=src_ap, in0=src_ap, scalar1=1.0,
                    scalar2=None, op0=OP.mult, op1=OP.max,
                    accum_out=mmax[:, c:c + 1])
                nc.vector.tensor_scalar(
                    out=src_ap, in0=src_ap, scalar1=1.0,
                    scalar2=None, op0=OP.mult, op1=OP.min,
                    accum_out=mmin[:, c:c + 1])
                mmcol[0] = c + 1

            for s0, s1 in CH2:
                mmts(r5[:, RB[2] + s0 * SC:RB[2] + s1 * SC])
            for s0, s1 in CH:
                mmts(r5[:, RB[3] + s0 * SC:RB[3] + s1 * SC])
            mmts(r5[:, RB[4]:RB[4] + 7 * SC])
            mmts(r5[:, RB[4] + 7 * SC:RB[4] + 14 * SC])
            for ch in a1ch[:2]:
                mmts(ch[:])
            mmts(r5[:, RB[5]:RB[5] + 7 * SC])
            mmts(r5[:, RB[5] + 7 * SC:RB[5] + 14 * SC])
            for ch in a1ch[2:6]:
                mmts(ch[:])
            mmts(r5[:, RB[0]:RB[0] + 7 * SC])
            mmts(a1ch[6][:])
            mmts(r5[:, RB[0] + 7 * SC:RB[0] + 14 * SC])
            nused = mmcol[0]
            nc.vector.memset(mmax[:, nused:32], -3.0e38)
            nc.vector.memset(mmin[:, nused:32], 3.0e38)

            # raw gen_L + gen_A + gen_W sum on PE (42 identity matmuls),
            # interleaved in chunk-arrival order. Reads race nothing: the
            # in-place minmax writes back identical bits.
            p_law = psum.tile([128, 3 * W], F32)
            mms = []
            for s0, s1 in CH:
                for a in (2, 3, 5):
                    mms += [(a, s) for s in range(s0, s1)]
            for j, (a, s) in enumerate(mms):
                rhs = r5[:, RB[a] + s * SC:RB[a] + s * SC + 3 * W]
                nc.tensor.matmul(p_law[:], ident[:], rhs,
                                 start=(j == 0), stop=(j == len(mms) - 1))

            # fold minmax partials: mm2 = [rowmax, -rowmin]
            nc.vector.tensor_reduce(out=mm2[:, 0:1], in_=mmax[:],
                                    axis=AX.X, op=OP.max)
            nc.vector.tensor_reduce(out=mtmp[:], in_=mmin[:],
                                    axis=AX.X, op=OP.min)
            nc.vector.tensor_scalar(out=mm2[:, 1:2], in0=mtmp[:], scalar1=-1.0,
                                    scalar2=None, op0=OP.mult)
            p_tt = psum.tile([2, 256], F32)
            p_t = p_tt[:, 0:128]
            nc.tensor.transpose(p_t, mm2[:], ident[:])
            nc.vector.memset(gpad[:], -3.0e38)
            nc.vector.tensor_reduce(out=gpad[0:2, 0:1], in_=p_t, axis=AX.X,
                                    op=OP.max)

            # ---------------- collective 1 ----------------
            nc.sync.dma_start(out=cin1[:], in_=gpad[:])
            if NO_COLL:
                nc.gpsimd.dma_start(
                    out=gsb[:], in_=cin1[0:2, 0:1].rearrange("p o -> o p"))
            else:
                nc.gpsimd.collective_compute(
                    "AllReduce", OP.max, replica_groups=[list(range(N_CORES))],
                    ins=[cin1.opt()], outs=[cout1.opt()])
                nc.sync.dma_start(
                    out=gsb[:], in_=cout1[0:2, 0:1].rearrange("p o -> o p"))

            # overlap collective-1 latency: LAW psum drain + GT_norm
            nc.vector.tensor_copy(law[:], p_law[:])
            nc.vector.tensor_scalar(out=gtn[:], in0=gtn[:], scalar1=0.5,
                                    scalar2=0.5, op0=OP.mult, op1=OP.add)

            # ---------------- threshold row ----------------
            # rowb: 0:t08 1:nmn 2:nt02 3:nt04 4:nt06 5:nt08
            #       6:sgscale 7:sgbias 8:invd 9:g3bias
            gmax, ngmn = gsb[:, 0:1], gsb[:, 1:2]
            nc.vector.tensor_tensor(out=dd[:], in0=gmax, in1=ngmn, op=OP.add)
            nc.vector.reciprocal(invd[:], dd[:])
            nc.vector.tensor_copy(rowb[:, 1:2], ngmn)
            for ck, col in ((0.2, 2), (0.4, 3), (0.6, 4), (0.8, 5)):
                nc.vector.tensor_scalar(out=rowb[:, col:col + 1], in0=dd[:],
                                        scalar1=-ck, scalar2=ngmn,
                                        op0=OP.mult, op1=OP.add)
            nc.vector.tensor_scalar(out=rowb[:, 0:1], in0=rowb[:, 5:6],
                                    scalar1=-1.0, scalar2=None, op0=OP.mult)
            nc.vector.tensor_scalar(out=rowb[:, 6:7], in0=invd[:], scalar1=10.0,
                                    scalar2=None, op0=OP.mult)
            nc.vector.tensor_tensor(out=rtmp[:], in0=ngmn, in1=invd[:],
                                    op=OP.mult)  # (-mn)/d
            nc.vector.tensor_scalar(out=rowb[:, 7:8], in0=rtmp[:], scalar1=10.0,
                                    scalar2=-9.0, op0=OP.mult, op1=OP.add)
            nc.vector.tensor_copy(rowb[:, 8:9], invd[:])
            nc.vector.tensor_scalar(out=rowb[:, 9:10], in0=rtmp[:],
                                    scalar1=42.0, scalar2=None, op0=OP.mult)
            p_misc = psum.tile([128, 16], F32)
            p_b = p_misc[:, 0:10]
            nc.tensor.matmul(p_b, ones_r[:], rowb[:, 0:10],
                             start=True, stop=True)
            nc.vector.tensor_copy(thrb[:, 0:10], p_b)
            T08, NMN, NT02, NT04, NT06, NT08 = (thrb[:, i:i + 1]
                                                for i in range(6))
            SGS, SGB, INVD, G3B = (thrb[:, i:i + 1] for i in range(6, 10))

            # law := G3 = (law - 42*mn)/d  (in place; raw sum is dead)
            nc.vector.tensor_scalar(out=law[:], in0=law[:], scalar1=INVD,
                                    scalar2=G3B, op0=OP.mult, op1=OP.add)

            # ---------------- phase 2: counts ----------
            rgb0 = r5[:, RB[0]:RB[0] + AT].rearrange(
                "h (s c w) -> h s c w", s=SQE, c=4)[:, :, 0:3, :]
            a0a = r5[:, RB[0]:RB[0] + AT].rearrange(
                "h (s c w) -> h s c w", s=SQE, c=4)[:, :, 3, :]
            # alpha mask (0/1) for gating
            nc.vector.tensor_scalar(
                out=alm[:].rearrange("h (s w) -> h s w", w=W), in0=a0a,
                scalar1=T08, scalar2=None, op0=OP.is_gt)
            almb = alm[:].rearrange("h (s w) -> h s w", w=W)[:, :, None, :]

            def sgn(out_t, nbias):  # sign(x - t) on ACT, exact +-1/0
                nc.scalar.activation(
                    out_t[:].rearrange("h (s c w) -> h s c w", s=SQE, c=3),
                    rgb0, AF.Sign, bias=nbias)

            def gate(t):  # in-place: t *= alpha mask (broadcast over c)
                v = t[:].rearrange("h (s c w) -> h s c w", s=SQE, c=3)
                nc.vector.tensor_tensor(
                    out=v, in0=v, in1=almb.to_broadcast([128, SQE, 3, W]),
                    op=OP.mult)

            def fold(dst_col, t, ng=42):
                # sum over w per group: bf16 TT half-fold tree to width 8
                # (2x mode; integer partial sums <= 16 stay exact), then a
                # small multi-dim reduce. Result exact (|sums| <= 128).
                v = t[:].rearrange("h (g w) -> h g w", w=W)
                for wdt in (64, 32, 16, 8):
                    nc.vector.tensor_tensor(
                        out=v[:, :, 0:wdt], in0=v[:, :, 0:wdt],
                        in1=v[:, :, wdt:2 * wdt], op=OP.add)
                with nc.allow_low_precision("exact: integer sums <= 128"):
                    nc.vector.tensor_reduce(
                        out=cnt[:, dst_col:dst_col + ng],
                        in_=v[:, :, 0:8], axis=AX.X, op=OP.add)

            # ACT computes all five indicators as signs (+-1; sign(x-mn)
            # is 0/1 since x >= mn); DVE only gates and folds. Buffers
            # rotate so neither engine idles on the other.
            sgn(ib0, NT08)
            sgn(ib1, NMN)
            fold(0, ib0)         # S8
            sgn(ib0, NT02)
            gate(ib1)
            fold(126, ib1)       # Pmn
            sgn(ib1, NT04)
            gate(ib0)
            fold(210, alm, SQE)  # N_A per slot (tree destroys the mask)
            fold(168, ib0)       # P02
            sgn(ib0, NT06)
            fold(42, ib1)        # S4
            a4r = r5[:, RB[4]:RB[4] + AT].rearrange(
                "h (s c w) -> h s c w", s=SQE, c=4)[:, :, 0:3, :]
            sig4 = ib1[:].rearrange("h (s c w) -> h s c w", s=SQE, c=3)
            nc.scalar.activation(sig4, a4r, AF.Sigmoid, bias=SGB, scale=SGS)
            fold(84, ib0)        # S6
            # region product into alm (mask is dead after the gates)
            reg3 = alm[:].rearrange("h (s w) -> h s w", w=W)
            nc.vector.tensor_tensor(out=reg3, in0=sig4[:, :, 0, :],
                                    in1=sig4[:, :, 1, :], op=OP.mult)
            nc.vector.tensor_tensor(out=reg3, in0=reg3,
                                    in1=sig4[:, :, 2, :], op=OP.mult)

            # partition fold: crow[1,210] = ones^T @ cnt
            p_cnt = psum.tile([1, 224], F32)
            nc.tensor.matmul(p_cnt[:], ones_cb[:], cnt[:], start=True,
                             stop=True)
            nc.vector.tensor_copy(crow[:], p_cnt[:])

            # ---------------- type_list on the [1,42] row ----------------
            # doubled counts: C2d = S8+N, C1d = S4-S6, C0d = 2*(Pmn-P02)
            S8, S4, S6 = crow[:, 0:42], crow[:, 42:84], crow[:, 84:126]
            PMN, P02 = crow[:, 126:168], crow[:, 168:210]
            NA = crow[:, 210:224]
            C2d, C1d, C0d = drow[:, 0:42], drow[:, 42:84], drow[:, 84:126]
            nc.vector.tensor_scalar(out=C2d, in0=S8, scalar1=NBLK, scalar2=None,
                                    op0=OP.add)
            nc.vector.tensor_tensor(out=C1d, in0=S4, in1=S6, op=OP.subtract)
            nc.vector.scalar_tensor_tensor(out=C0d, in0=PMN, scalar=2.0,
                                           in1=P02, op0=OP.mult,
                                           op1=OP.subtract)
            nab = NA.rearrange("h (s o) -> h s o", o=1)
            nc.vector.tensor_tensor(
                out=C0d.rearrange("h (s c) -> h s c", c=3),
                in0=C0d.rearrange("h (s c) -> h s c", c=3),
                in1=nab.to_broadcast([1, SQE, 3]), op=OP.subtract)

            t1, t2, b2 = rows[:, 84:126], rows[:, 126:168], rows[:, 168:210]
            nb2, vals = rows[:, 210:252], rows[:, 252:294]
            nc.vector.tensor_tensor(out=t1, in0=C2d, in1=C1d, op=OP.is_gt)
            nc.vector.tensor_tensor(out=t2, in0=C2d, in1=C0d, op=OP.is_gt)
            nc.vector.tensor_tensor(out=b2, in0=t1, in1=t2, op=OP.mult)
            nc.vector.tensor_scalar(out=nb2, in0=b2, scalar1=-1.0, scalar2=1.0,
                                    op0=OP.mult, op1=OP.add)
            nc.vector.tensor_tensor(out=t1, in0=C1d, in1=C0d, op=OP.is_gt)
            nc.vector.tensor_tensor(out=t2, in0=nb2, in1=t1, op=OP.mult)  # b1
            nc.vector.scalar_tensor_tensor(out=vals, in0=t2, scalar=0.5,
                                           in1=b2, op0=OP.mult, op1=OP.add)
            vv = vals.rearrange("h (s c) -> h s c", c=3)
            v0, v1, v2 = vv[:, :, 0], vv[:, :, 1], vv[:, :, 2]
            sv, s6 = srow[:, 0:14], srow[:, 14:28]
            qq, q2 = srow[:, 28:42], srow[:, 42:56]
            e3, band = srow[:, 56:70], srow[:, 70:84]
            etmp, mem = srow[:, 84:98], srow[:, 98:112]
            nc.vector.tensor_tensor(out=sv, in0=v0, in1=v1, op=OP.add)
            nc.vector.tensor_tensor(out=sv, in0=sv, in1=v2, op=OP.add)
            nc.vector.tensor_scalar(out=s6, in0=sv, scalar1=2.0, scalar2=None,
                                    op0=OP.mult)
            nc.vector.scalar_tensor_tensor(out=qq, in0=v0, scalar=2.0, in1=v1,
                                           op0=OP.mult, op1=OP.add)
            nc.vector.scalar_tensor_tensor(out=q2, in0=qq, scalar=2.0, in1=v2,
                                           op0=OP.mult, op1=OP.add)
            nc.vector.tensor_scalar(out=qq, in0=q2, scalar1=2.0, scalar2=None,
                                    op0=OP.mult)
            nc.vector.tensor_scalar(out=mem, in0=s6, scalar1=0.0, scalar2=None,
                                    op0=OP.is_equal)
            for sval in (4.0, 6.0):
                nc.vector.tensor_scalar(out=etmp, in0=s6, scalar1=sval,
                                        scalar2=None, op0=OP.is_equal)
                nc.vector.tensor_tensor(out=mem, in0=mem, in1=etmp, op=OP.add)
            nc.vector.tensor_scalar(out=e3, in0=s6, scalar1=3.0, scalar2=None,
                                    op0=OP.is_equal)
            nc.vector.tensor_scalar(out=band, in0=qq, scalar1=7.0, scalar2=None,
                                    op0=OP.is_ge)
            nc.vector.tensor_scalar(out=etmp, in0=qq, scalar1=9.0, scalar2=None,
                                    op0=OP.is_le)
            nc.vector.tensor_tensor(out=band, in0=band, in1=etmp, op=OP.mult)
            nc.vector.tensor_tensor(out=e3, in0=e3, in1=band, op=OP.mult)
            nc.vector.tensor_tensor(out=mem, in0=mem, in1=e3, op=OP.add)
            # type = vals * member (broadcast member over c)
            tyrow = rows[:, 84:126]  # reuse
            nc.vector.tensor_tensor(
                out=tyrow.rearrange("h (s c) -> h s c", c=3), in0=vv,
                in1=mem[:, :, None].to_broadcast([1, 14, 3]), op=OP.mult)
            p_ty = psum.tile([128, 42], F32)
            nc.tensor.matmul(p_ty[:], ones_r[:], tyrow, start=True, stop=True)
            nc.vector.tensor_copy(typb[:], p_ty[:])

            # ---------------- weighted slot-sum (DVE stt chains) --------
            for c in range(3):
                nc.vector.tensor_scalar(
                    out=acc[:, c * W:(c + 1) * W], in0=alm[:, 0:W],
                    scalar1=typb[:, c:c + 1], scalar2=None, op0=OP.mult)
            for s in range(1, SQE):
                for c in range(3):
                    a_c = acc[:, c * W:(c + 1) * W]
                    nc.vector.scalar_tensor_tensor(
                        out=a_c, in0=alm[:, s * W:(s + 1) * W],
                        scalar=typb[:, s * 3 + c:s * 3 + c + 1], in1=a_c,
                        op0=OP.mult, op1=OP.add)

            # ---------------- collective 2 (region min/max) ----------------
            nc.vector.tensor_reduce(out=rmm2[:, 0:1], in_=acc[:], axis=AX.X,
                                    op=OP.max)
            nc.vector.tensor_reduce(out=mtmp[:], in_=acc[:], axis=AX.X,
                                    op=OP.min)
            nc.vector.tensor_scalar(out=rmm2[:, 1:2], in0=mtmp[:], scalar1=-1.0,
                                    scalar2=None, op0=OP.mult)
            p_t2 = p_tt[:, 128:256]
            nc.tensor.transpose(p_t2, rmm2[:], ident[:])
            nc.vector.memset(rpad[:], -3.0e38)
            nc.vector.tensor_reduce(out=rpad[0:2, 0:1], in_=p_t2, axis=AX.X,
                                    op=OP.max)
            nc.sync.dma_start(out=cin2[:], in_=rpad[:])
            if NO_COLL:
                nc.gpsimd.dma_start(
                    out=rsb[:], in_=cin2[0:2, 0:1].rearrange("p o -> o p"))
            else:
                nc.gpsimd.collective_compute(
                    "AllReduce", OP.max, replica_groups=[list(range(N_CORES))],
                    ins=[cin2.opt()], outs=[cout2.opt()])
                nc.gpsimd.dma_start(
                    out=rsb[:], in_=cout2[0:2, 0:1].rearrange("p o -> o p"))

            nc.vector.tensor_tensor(out=dd[:], in0=rsb[:, 0:1], in1=rsb[:, 1:2],
                                    op=OP.add)
            nc.vector.reciprocal(rinvd[:], dd[:])
            nc.vector.tensor_copy(rrow[:, 0:1], rinvd[:])
            nc.vector.tensor_tensor(out=rrow[:, 1:2], in0=rsb[:, 1:2],
                                    in1=rinvd[:], op=OP.mult)
            p_b2 = p_misc[:, 10:12]
            nc.tensor.matmul(p_b2, ones_r[:], rrow[:], start=True, stop=True)
            nc.vector.tensor_copy(rcp[:], p_b2)

            # ---------------- composite + SSE ----------------
            nc.vector.tensor_scalar(out=law[:], in0=law[:], scalar1=rcp[:, 1:2],
                                    scalar2=None, op0=OP.add)
            nc.vector.scalar_tensor_tensor(out=law[:], in0=acc[:],
                                           scalar=rcp[:, 0:1], in1=law[:],
                                           op0=OP.mult, op1=OP.add)
            nc.vector.tensor_scalar(out=law[:], in0=law[:], scalar1=0.0,
                                    scalar2=1.0, op0=OP.max, op1=OP.min)
            nc.vector.tensor_tensor(out=law[:], in0=law[:], in1=gtn[:],
                                    op=OP.subtract)
            nc.vector.scalar_tensor_tensor(out=law[:], in0=law[:],
                                           scalar=1.0, in1=law[:], op0=OP.mult,
                                           op1=OP.mult, accum_out=ssecol[:])
            p_s = p_misc[0:1, 12:13]
            nc.tensor.matmul(p_s, ones_c[:], ssecol[:], start=True,
                             stop=True)
            nc.vector.tensor_copy(sse_sb[:], p_s)
            nc.sync.dma_start(out=sse.ap(), in_=sse_sb[:])

            # ---------------- debug outputs ----------------
            nc.sync.dma_start(out=dbg.ap()[:, 0:10], in_=rowb[:, 0:10])
            nc.sync.dma_start(out=dbg.ap()[:, 10:12], in_=rsb[:])
            nc.sync.dma_start(out=dbg.ap()[:, 12:14], in_=gsb[:])
            nc.sync.dma_start(out=dbg2.ap()[:, 0:126], in_=drow[:])
            nc.sync.dma_start(out=dbg2.ap()[:, 126:168], in_=tyrow)

    nc.finalize()
    return nc


_NC = None


def _get_nc():
    global _NC
    if _NC is None:
        _NC = build()
    return _NC


def run(gt_full, pred_full, trace=False):
    """Run the SPMD kernel on the full (8, ...) inputs. Returns
    (loss, BassKernelResults)."""
    nc = _get_nc()
    in_maps = [
        {"GT": np.ascontiguousarray(gt_full[i]),
         "Pred": np.ascontiguousarray(pred_full[i])}
        for i in range(N_CORES)
    ]
    res = run_bass_kernel_spmd(nc, in_maps, core_ids=list(range(N_CORES)),
                               trace=trace)
    total = sum(float(res.results[c]["sse"][0, 0]) for c in range(N_CORES))
    loss = np.float32(total / NPIX)
    return loss, res


def kernel(GT, Pred):
    gt_full = np.asarray(GT, dtype=np.float32)
    pred_full = np.asarray(Pred, dtype=np.float32)
    loss, _ = run(gt_full, pred_full, trace=False)
    return loss


if __name__ == "__main__":
    rng = np.random.default_rng(0)
    gt = rng.random((8, 4, H, W), dtype=np.float32)
    pr = rng.random((8, 6, SQE, 4, H, W), dtype=np.float32)
    print("loss:", kernel(gt, pr))


# revision 36
# speedup vs baseline: 1.1102x; 1.1102x over previous
"""Trainium2 Bass kernel for nn_ComposeImgLoss (8-core data-parallel).

Contract: kernel(**inputs) takes the FULL inputs
    GT   (8, 4, 128, 128) f32
    Pred (8, 6, 14, 4, 128, 128) f32
and returns the FULL scalar loss (f32), matching reference.reference().

Strategy (1 sample per core, 8 cores). Measured op modes on this HW:
plain tensor_scalar / tensor_copy are 2x (fp32), tensor_tensor is 2x in
bf16 only; all accum/reduce/STT/ACT paths are 1x; DMA is HBM-limited at
~264 GB/s/core (~84us for the 21MB Pred slice); first two collectives
are cold (~33/24us), later ones ~10us.

  phase 0: Pred DMA'd in 4-slot chunks alternating across both HW DGE
    rings (sync=SP, scalar=Activation) so completion semaphores fire
    every ~4us; two dummy AllReduces fire at t=0 to warm the CC cores.
  phase 1 (under the DMA shadow): min/max per chunk via in-place
    tensor_scalar identity + accum_out (elementwise out == input, so no
    scratch and no data change); gen_L+gen_A+gen_W summed on PE via
    identity-matmul accumulation.
  collective 1: AllReduce(max) of [max, -min].
  phase 2: thresholds broadcast via ones-matmul; count indicators:
    DVE is_gt (0/1) for the alpha-gated pair (mn, t02) + ACT Sign (+-1)
    for t08/t04/t06; per-(slot,chan) sums via bf16 TT half-fold trees
    (exact: integer partial sums <= 128); partition fold via one bf16
    ones-matmul; type_list vals/membership on a [1,42] row; sigmoid on
    ACT; weighted slot-sum on PE via scaled-identity PSUM accumulation.
  collective 2: AllReduce(max) of region-sum [max, -min]; composite =
    clip(region_contrib + G3, 0, 1); SSE via fused square+accum and a
    ones-matmul partition fold.
  host: loss = sum(sse_core) / (8*3*128*128)
"""

import numpy as np

import concourse.bass as bass
import concourse.bacc as bacc
import concourse.tile as tile
from concourse import mybir
from concourse.masks import make_identity
from concourse.bass_utils import run_bass_kernel_spmd

import os
NO_COLL = os.environ.get("NO_COLL", "0") == "1"
NO_WARM = os.environ.get("NO_WARM", "0") == "1"

F32 = mybir.dt.float32
BF16 = mybir.dt.bfloat16
OP = mybir.AluOpType
AF = mybir.ActivationFunctionType
AX = mybir.AxisListType

N_CORES = 8
SQE, H, W = 14, 128, 128
SC = 4 * W            # cols per slot (4 channels x 128 w)
AT = SQE * SC         # 7168 cols per attri
RES = [0, 2, 3, 4, 5]  # SBUF-resident attris; attri 1 is streamed
RB = {a: i * AT for i, a in enumerate(RES)}
NPIX = N_CORES * 3 * H * W  # denominator of the global mean
NBLK = float(H * W)         # pixels per (slot, chan) block


def build():
    nc = bacc.Bacc("TRN2", target_bir_lowering=False, debug=False,
                   num_devices=N_CORES)
    pred = nc.dram_tensor("Pred", [6, SQE, 4, H, W], F32, kind="ExternalInput")
    gt = nc.dram_tensor("GT", [4, H, W], F32, kind="ExternalInput")
    sse = nc.dram_tensor("sse", [1, 1], F32, kind="ExternalOutput")
    dbg = nc.dram_tensor("dbg", [1, 16], F32, kind="ExternalOutput")
    dbg2 = nc.dram_tensor("dbg2", [1, 256], F32, kind="ExternalOutput")

    with tile.TileContext(nc) as tc:
        with (
            tc.tile_pool(name="big", bufs=1) as big,
            tc.tile_pool(name="a1s", bufs=7) as a1s,
            tc.tile_pool(name="mid", bufs=1) as mid,
            tc.tile_pool(name="small", bufs=1) as small,
            tc.tile_pool(name="psum", bufs=1, space="PSUM") as psum,
            tc.tile_pool(name="dram", bufs=1, space="DRAM") as dram,
        ):
            # ---------------- tiles ----------------
            r5 = big.tile([128, 5 * AT], F32)          # 140KB/part
            ib0 = mid.tile([128, SQE * 3 * W], BF16)   # indicator bufs
            ib1 = mid.tile([128, SQE * 3 * W], BF16)
            alm = mid.tile([128, SQE * W], BF16)       # alpha mask -> region
            law = mid.tile([128, 3 * W], F32)          # raw L+A+W -> G3 -> SSE
            acc = mid.tile([128, 3 * W], BF16)         # weighted slot-sum
            gtn = mid.tile([128, 3 * W], F32)

            ident = small.tile([128, 128], F32)
            ones_c = small.tile([128, 1], F32)
            ones_cb = small.tile([128, 1], BF16)
            ones_r = small.tile([1, 128], F32)
            mmax = small.tile([128, 32], F32)          # per-chunk max cols
            mmin = small.tile([128, 32], F32)          # per-chunk min cols
            mm2 = small.tile([128, 2], F32)
            mtmp = small.tile([128, 1], F32)
            cnt = small.tile([128, 224], BF16)         # folded count sums
            gsb = small.tile([1, 2], F32)
            rsb = small.tile([1, 2], F32)
            rowb = small.tile([1, 16], F32)
            rtmp = small.tile([1, 1], F32)
            dd = small.tile([1, 1], F32)
            invd = small.tile([1, 1], F32)
            rinvd = small.tile([1, 1], F32)
            rrow = small.tile([1, 2], F32)
            crow = small.tile([1, 224], F32)
            drow = small.tile([1, 126], F32)           # C2d C1d C0d
            rows = small.tile([1, 300], F32)
            srow = small.tile([1, 112], F32)
            thrb = small.tile([128, 12], F32)
            typb = small.tile([128, 42], F32)
            rmm2 = small.tile([128, 2], F32)
            rcp = small.tile([128, 2], F32)
            ssecol = small.tile([128, 1], F32)
            sse_sb = small.tile([1, 1], F32)
            warm = small.tile([128, 1], F32)
            warm2 = small.tile([128, 1], F32)
            zrow = small.tile([16, 1], F32)

            cin1 = dram.tile([16, 1], F32)
            cout1 = dram.tile([16, 1], F32, addr_space="Shared")
            cin2 = dram.tile([16, 1], F32)
            cout2 = dram.tile([16, 1], F32, addr_space="Shared")
            cinw = dram.tile([16, 1], F32)
            coutw = dram.tile([16, 1], F32, addr_space="Shared")
            cinw2 = dram.tile([16, 1], F32)
            coutw2 = dram.tile([16, 1], F32, addr_space="Shared")
            cinw3 = dram.tile([16, 1], F32)
            coutw3 = dram.tile([16, 1], F32, addr_space="Shared")
            gpad = small.tile([16, 1], F32)
            rpad = small.tile([16, 1], F32)

            # ---------------- warmup collectives ----------------
            nc.vector.memset(zrow[:], 0.0)
            nc.gpsimd.dma_start(out=cinw[:], in_=zrow[:])
            if not (NO_COLL or NO_WARM):
                nc.gpsimd.collective_compute(
                    "AllReduce", OP.max, replica_groups=[list(range(N_CORES))],
                    ins=[cinw.opt()], outs=[coutw.opt()])
                nc.gpsimd.dma_start(out=cinw2[:], in_=zrow[:])
                nc.gpsimd.collective_compute(
                    "AllReduce", OP.max, replica_groups=[list(range(N_CORES))],
                    ins=[cinw2.opt()], outs=[coutw2.opt()])
                nc.gpsimd.dma_start(out=cinw3[:], in_=zrow[:])
                nc.gpsimd.collective_compute(
                    "AllReduce", OP.max, replica_groups=[list(range(N_CORES))],
                    ins=[cinw3.opt()], outs=[coutw3.opt()])

            # ---------------- phase 0: DMA fan-out ----------------
            pr = pred.ap()

            def load(engine, a, s0, s1):
                off = RB[a] + s0 * SC
                engine.dma_start(
                    out=r5[:, off:off + (s1 - s0) * SC].rearrange(
                        "h (s c w) -> h s c w", s=s1 - s0, c=4),
                    in_=pr[a][s0:s1].rearrange("s c h w -> h s c w"))

            # 4-slot chunks alternating across the two rings so completion
            # semaphores fire every ~4us and DVE minmax tracks arrivals.
            CH = [(0, 4), (4, 8), (8, 12), (12, 14)]
            pa1 = pr[1].rearrange("s c h w -> h s c w")
            a1ch = []
            # a2/a3 in fine 4-slot chunks (DVE ramp-up); then a4/a5/a0
            # as halves split across BOTH rings (each ring sustains only
            # ~118GB/s when both run, so halves land ~2x sooner than
            # full-attri loads), with a1 chunks interleaved between them
            # so arrivals track DVE's minmax consumption.
            def load_a1(k, eng):
                ch = a1s.tile([128, 1024], F32, tag="a1chunk")
                eng.dma_start(
                    out=ch[:].rearrange("h (s c w) -> h s c w", s=2, c=4),
                    in_=pa1[:, 2 * k:2 * k + 2])
                a1ch.append(ch)

            for ai, a in enumerate((2, 3)):
                for q, (s0, s1) in enumerate(CH):
                    load(nc.sync if (q + ai) % 2 == 0 else nc.scalar,
                         a, s0, s1)
            load(nc.sync, 4, 0, 7)
            load(nc.scalar, 4, 7, SQE)
            load_a1(0, nc.sync)
            load_a1(1, nc.scalar)
            load(nc.sync, 5, 0, 7)
            load(nc.scalar, 5, 7, SQE)
            load_a1(2, nc.sync)
            load_a1(3, nc.scalar)
            load(nc.sync, 0, 0, 7)
            load(nc.scalar, 0, 7, SQE)
            load_a1(4, nc.sync)
            load_a1(5, nc.scalar)
            load_a1(6, nc.scalar)
            nc.sync.dma_start(out=gtn[:].rearrange("h (c w) -> h c w", w=W),
                              in_=gt.ap()[0:3].rearrange("c h w -> h c w"))

            # ---------------- constants ----------------
            make_identity(nc, ident[:])
            nc.vector.memset(ones_c[:], 1.0)
            nc.vector.memset(ones_cb[:], 1.0)
            nc.vector.memset(ones_r[:], 1.0)
            # ACT table warmups: sigmoid then sign, so the Sign table is
            # resident when phase-2 counts start.
            nc.scalar.activation(warm[:], ones_c[:], AF.Sigmoid)
            nc.scalar.activation(warm2[:], ones_c[:], AF.Sign)

            # ---------------- phase 1: min/max + LAW ----------------
            # In-place tensor_scalar identity (x*1.0 == x, bit-exact) +
            # accum_out; no scratch tile, no value change. 1x mode, but
            # chunks track DMA arrivals so DVE streams behind the load.
            mmcol = [0]

            def mmts(src_ap):
                c = mmcol[0]
                nc.vector.tensor_scalar(
                    out=src_ap, in0=src_ap, scalar1=1.0,
                    scalar2=None, op0=OP.mult, op1=OP.max,
                    accum_out=mmax[:, c:c + 1])
                nc.vector.tensor_scalar(
                    out=src_ap, in0=src_ap, scalar1=1.0,
                    scalar2=None, op0=OP.mult, op1=OP.min,
                    accum_out=mmin[:, c:c + 1])
                mmcol[0] = c + 1

            for a in (2, 3):
                for s0, s1 in CH:
                    mmts(r5[:, RB[a] + s0 * SC:RB[a] + s1 * SC])
            mmts(r5[:, RB[4]:RB[4] + 7 * SC])
            mmts(r5[:, RB[4] + 7 * SC:RB[4] + 14 * SC])
            for ch in a1ch[:2]:
                mmts(ch[:])
            mmts(r5[:, RB[5]:RB[5] + 7 * SC])
            mmts(r5[:, RB[5] + 7 * SC:RB[5] + 14 * SC])
            for ch in a1ch[2:4]:
                mmts(ch[:])
            mmts(r5[:, RB[0]:RB[0] + 7 * SC])
            mmts(r5[:, RB[0] + 7 * SC:RB[0] + 14 * SC])
            for ch in a1ch[4:]:
                mmts(ch[:])
            nused = mmcol[0]
            nc.vector.memset(mmax[:, nused:32], -3.0e38)
            nc.vector.memset(mmin[:, nused:32], 3.0e38)

            # raw gen_L + gen_A + gen_W sum on PE (42 identity matmuls),
            # interleaved in chunk-arrival order. Reads race nothing: the
            # in-place minmax writes back identical bits.
            p_law = psum.tile([128, 3 * W], F32)
            mms = []
            for s0, s1 in CH:
                for a in (2, 3, 5):
                    mms += [(a, s) for s in range(s0, s1)]
            for j, (a, s) in enumerate(mms):
                rhs = r5[:, RB[a] + s * SC:RB[a] + s * SC + 3 * W]
                nc.tensor.matmul(p_law[:], ident[:], rhs,
                                 start=(j == 0), stop=(j == len(mms) - 1))

            # fold minmax partials: mm2 = [rowmax, -rowmin]
            nc.vector.tensor_reduce(out=mm2[:, 0:1], in_=mmax[:],
                                    axis=AX.X, op=OP.max)
            nc.vector.tensor_reduce(out=mtmp[:], in_=mmin[:],
                                    axis=AX.X, op=OP.min)
            nc.vector.tensor_scalar(out=mm2[:, 1:2], in0=mtmp[:], scalar1=-1.0,
                                    scalar2=None, op0=OP.mult)
            p_tt = psum.tile([2, 256], F32)
            p_t = p_tt[:, 0:128]
            nc.tensor.transpose(p_t, mm2[:], ident[:])
            nc.vector.memset(gpad[:], -3.0e38)
            nc.vector.tensor_reduce(out=gpad[0:2, 0:1], in_=p_t, axis=AX.X,
                                    op=OP.max)

            # ---------------- collective 1 ----------------
            nc.sync.dma_start(out=cin1[:], in_=gpad[:])
            if NO_COLL:
                nc.gpsimd.dma_start(
                    out=gsb[:], in_=cin1[0:2, 0:1].rearrange("p o -> o p"))
            else:
                nc.gpsimd.collective_compute(
                    "AllReduce", OP.max, replica_groups=[list(range(N_CORES))],
                    ins=[cin1.opt()], outs=[cout1.opt()])
                nc.sync.dma_start(
                    out=gsb[:], in_=cout1[0:2, 0:1].rearrange("p o -> o p"))

            # overlap collective-1 latency: LAW psum drain + GT_norm
            nc.vector.tensor_copy(law[:], p_law[:])
            nc.vector.tensor_scalar(out=gtn[:], in0=gtn[:], scalar1=0.5,
                                    scalar2=0.5, op0=OP.mult, op1=OP.add)

            # ---------------- threshold row ----------------
            # rowb: 0:t08 1:nmn 2:nt02 3:nt04 4:nt06 5:nt08
            #       6:sgscale 7:sgbias 8:invd 9:g3bias
            gmax, ngmn = gsb[:, 0:1], gsb[:, 1:2]
            nc.vector.tensor_tensor(out=dd[:], in0=gmax, in1=ngmn, op=OP.add)
            nc.vector.reciprocal(invd[:], dd[:])
            nc.vector.tensor_copy(rowb[:, 1:2], ngmn)
            for ck, col in ((0.2, 2), (0.4, 3), (0.6, 4), (0.8, 5)):
                nc.vector.tensor_scalar(out=rowb[:, col:col + 1], in0=dd[:],
                                        scalar1=-ck, scalar2=ngmn,
                                        op0=OP.mult, op1=OP.add)
            nc.vector.tensor_scalar(out=rowb[:, 0:1], in0=rowb[:, 5:6],
                                    scalar1=-1.0, scalar2=None, op0=OP.mult)
            nc.vector.tensor_scalar(out=rowb[:, 6:7], in0=invd[:], scalar1=10.0,
                                    scalar2=None, op0=OP.mult)
            nc.vector.tensor_tensor(out=rtmp[:], in0=ngmn, in1=invd[:],
                                    op=OP.mult)  # (-mn)/d
            nc.vector.tensor_scalar(out=rowb[:, 7:8], in0=rtmp[:], scalar1=10.0,
                                    scalar2=-9.0, op0=OP.mult, op1=OP.add)
            nc.vector.tensor_copy(rowb[:, 8:9], invd[:])
            nc.vector.tensor_scalar(out=rowb[:, 9:10], in0=rtmp[:],
                                    scalar1=42.0, scalar2=None, op0=OP.mult)
            p_misc = psum.tile([128, 16], F32)
            p_b = p_misc[:, 0:10]
            nc.tensor.matmul(p_b, ones_r[:], rowb[:, 0:10],
                             start=True, stop=True)
            nc.vector.tensor_copy(thrb[:, 0:10], p_b)
            T08, NMN, NT02, NT04, NT06, NT08 = (thrb[:, i:i + 1]
                                                for i in range(6))
            SGS, SGB, INVD, G3B = (thrb[:, i:i + 1] for i in range(6, 10))

            # law := G3 = (law - 42*mn)/d  (in place; raw sum is dead)
            nc.vector.tensor_scalar(out=law[:], in0=law[:], scalar1=INVD,
                                    scalar2=G3B, op0=OP.mult, op1=OP.add)

            # ---------------- phase 2: counts ----------
            rgb0 = r5[:, RB[0]:RB[0] + AT].rearrange(
                "h (s c w) -> h s c w", s=SQE, c=4)[:, :, 0:3, :]
            a0a = r5[:, RB[0]:RB[0] + AT].rearrange(
                "h (s c w) -> h s c w", s=SQE, c=4)[:, :, 3, :]
            # alpha mask (0/1) for gating
            nc.vector.tensor_scalar(
                out=alm[:].rearrange("h (s w) -> h s w", w=W), in0=a0a,
                scalar1=T08, scalar2=None, op0=OP.is_gt)
            almb = alm[:].rearrange("h (s w) -> h s w", w=W)[:, :, None, :]

            def sgn(out_t, nbias):  # sign(x - t) on ACT, exact +-1/0
                nc.scalar.activation(
                    out_t[:].rearrange("h (s c w) -> h s c w", s=SQE, c=3),
                    rgb0, AF.Sign, bias=nbias)

            def gate(t):  # in-place: t *= alpha mask (broadcast over c)
                v = t[:].rearrange("h (s c w) -> h s c w", s=SQE, c=3)
                nc.vector.tensor_tensor(
                    out=v, in0=v, in1=almb.to_broadcast([128, SQE, 3, W]),
                    op=OP.mult)

            def fold(dst_col, t, ng=42):
                # sum over w per group: bf16 TT half-fold tree to width 8
                # (2x mode; integer partial sums <= 16 stay exact), then a
                # small multi-dim reduce. Result exact (|sums| <= 128).
                v = t[:].rearrange("h (g w) -> h g w", w=W)
                for wdt in (64, 32, 16, 8):
                    nc.vector.tensor_tensor(
                        out=v[:, :, 0:wdt], in0=v[:, :, 0:wdt],
                        in1=v[:, :, wdt:2 * wdt], op=OP.add)
                with nc.allow_low_precision("exact: integer sums <= 128"):
                    nc.vector.tensor_reduce(
                        out=cnt[:, dst_col:dst_col + ng],
                        in_=v[:, :, 0:8], axis=AX.X, op=OP.add)

            # ACT computes all five indicators as signs (+-1; sign(x-mn)
            # is 0/1 since x >= mn); DVE only gates and folds. Buffers
            # rotate so neither engine idles on the other.
            sgn(ib0, NT08)
            sgn(ib1, NMN)
            fold(0, ib0)         # S8
            sgn(ib0, NT02)
            gate(ib1)
            fold(126, ib1)       # Pmn
            sgn(ib1, NT04)
            gate(ib0)
            fold(210, alm, SQE)  # N_A per slot (tree destroys the mask)
            fold(168, ib0)       # P02
            sgn(ib0, NT06)
            fold(42, ib1)        # S4
            a4r = r5[:, RB[4]:RB[4] + AT].rearrange(
                "h (s c w) -> h s c w", s=SQE, c=4)[:, :, 0:3, :]
            sig4 = ib1[:].rearrange("h (s c w) -> h s c w", s=SQE, c=3)
            nc.scalar.activation(sig4, a4r, AF.Sigmoid, bias=SGB, scale=SGS)
            fold(84, ib0)        # S6
            # region product into alm (mask is dead after the gates)
            reg3 = alm[:].rearrange("h (s w) -> h s w", w=W)
            nc.vector.tensor_tensor(out=reg3, in0=sig4[:, :, 0, :],
                                    in1=sig4[:, :, 1, :], op=OP.mult)
            nc.vector.tensor_tensor(out=reg3, in0=reg3,
                                    in1=sig4[:, :, 2, :], op=OP.mult)

            # partition fold: crow[1,210] = ones^T @ cnt
            p_cnt = psum.tile([1, 224], F32)
            nc.tensor.matmul(p_cnt[:], ones_cb[:], cnt[:], start=True,
                             stop=True)
            nc.vector.tensor_copy(crow[:], p_cnt[:])

            # ---------------- type_list on the [1,42] row ----------------
            # doubled counts: C2d = S8+N, C1d = S4-S6, C0d = 2*(Pmn-P02)
            S8, S4, S6 = crow[:, 0:42], crow[:, 42:84], crow[:, 84:126]
            PMN, P02 = crow[:, 126:168], crow[:, 168:210]
            NA = crow[:, 210:224]
            C2d, C1d, C0d = drow[:, 0:42], drow[:, 42:84], drow[:, 84:126]
            nc.vector.tensor_scalar(out=C2d, in0=S8, scalar1=NBLK, scalar2=None,
                                    op0=OP.add)
            nc.vector.tensor_tensor(out=C1d, in0=S4, in1=S6, op=OP.subtract)
            nc.vector.scalar_tensor_tensor(out=C0d, in0=PMN, scalar=2.0,
                                           in1=P02, op0=OP.mult,
                                           op1=OP.subtract)
            nab = NA.rearrange("h (s o) -> h s o", o=1)
            nc.vector.tensor_tensor(
                out=C0d.rearrange("h (s c) -> h s c", c=3),
                in0=C0d.rearrange("h (s c) -> h s c", c=3),
                in1=nab.to_broadcast([1, SQE, 3]), op=OP.subtract)

            t1, t2, b2 = rows[:, 84:126], rows[:, 126:168], rows[:, 168:210]
            nb2, vals = rows[:, 210:252], rows[:, 252:294]
            nc.vector.tensor_tensor(out=t1, in0=C2d, in1=C1d, op=OP.is_gt)
            nc.vector.tensor_tensor(out=t2, in0=C2d, in1=C0d, op=OP.is_gt)
            nc.vector.tensor_tensor(out=b2, in0=t1, in1=t2, op=OP.mult)
            nc.vector.tensor_scalar(out=nb2, in0=b2, scalar1=-1.0, scalar2=1.0,
                                    op0=OP.mult, op1=OP.add)
            nc.vector.tensor_tensor(out=t1, in0=C1d, in1=C0d, op=OP.is_gt)
            nc.vector.tensor_tensor(out=t2, in0=nb2, in1=t1, op=OP.mult)  # b1
            nc.vector.scalar_tensor_tensor(out=vals, in0=t2, scalar=0.5,
                                           in1=b2, op0=OP.mult, op1=OP.add)
            vv = vals.rearrange("h (s c) -> h s c", c=3)
            v0, v1, v2 = vv[:, :, 0], vv[:, :, 1], vv[:, :, 2]
            sv, s6 = srow[:, 0:14], srow[:, 14:28]
            qq, q2 = srow[:, 28:42], srow[:, 42:56]
            e3, band = srow[:, 56:70], srow[:, 70:84]
            etmp, mem = srow[:, 84:98], srow[:, 98:112]
            nc.vector.tensor_tensor(out=sv, in0=v0, in1=v1, op=OP.add)
            nc.vector.tensor_tensor(out=sv, in0=sv, in1=v2, op=OP.add)
            nc.vector.tensor_scalar(out=s6, in0=sv, scalar1=2.0, scalar2=None,
                                    op0=OP.mult)
            nc.vector.scalar_tensor_tensor(out=qq, in0=v0, scalar=2.0, in1=v1,
                                           op0=OP.mult, op1=OP.add)
            nc.vector.scalar_tensor_tensor(out=q2, in0=qq, scalar=2.0, in1=v2,
                                           op0=OP.mult, op1=OP.add)
            nc.vector.tensor_scalar(out=qq, in0=q2, scalar1=2.0, scalar2=None,
                                    op0=OP.mult)
            nc.vector.tensor_scalar(out=mem, in0=s6, scalar1=0.0, scalar2=None,
                                    op0=OP.is_equal)
            for sval in (4.0, 6.0):
                nc.vector.tensor_scalar(out=etmp, in0=s6, scalar1=sval,
                                        scalar2=None, op0=OP.is_equal)
                nc.vector.tensor_tensor(out=mem, in0=mem, in1=etmp, op=OP.add)
            nc.vector.tensor_scalar(out=e3, in0=s6, scalar1=3.0, scalar2=None,
                                    op0=OP.is_equal)
            nc.vector.tensor_scalar(out=band, in0=qq, scalar1=7.0, scalar2=None,
                                    op0=OP.is_ge)
            nc.vector.tensor_scalar(out=etmp, in0=qq, scalar1=9.0, scalar2=None,
                                    op0=OP.is_le)
            nc.vector.tensor_tensor(out=band, in0=band, in1=etmp, op=OP.mult)
            nc.vector.tensor_tensor(out=e3, in0=e3, in1=band, op=OP.mult)
            nc.vector.tensor_tensor(out=mem, in0=mem, in1=e3, op=OP.add)
            # type = vals * member (broadcast member over c)
            tyrow = rows[:, 84:126]  # reuse
            nc.vector.tensor_tensor(
                out=tyrow.rearrange("h (s c) -> h s c", c=3), in0=vv,
                in1=mem[:, :, None].to_broadcast([1, 14, 3]), op=OP.mult)
            p_ty = psum.tile([128, 42], F32)
            nc.tensor.matmul(p_ty[:], ones_r[:], tyrow, start=True, stop=True)
            nc.vector.tensor_copy(typb[:], p_ty[:])

            # ---------------- weighted slot-sum (DVE stt chains) --------
            for c in range(3):
                nc.vector.tensor_scalar(
                    out=acc[:, c * W:(c + 1) * W], in0=alm[:, 0:W],
                    scalar1=typb[:, c:c + 1], scalar2=None, op0=OP.mult)
            for s in range(1, SQE):
                for c in range(3):
                    a_c = acc[:, c * W:(c + 1) * W]
                    nc.vector.scalar_tensor_tensor(
                        out=a_c, in0=alm[:, s * W:(s + 1) * W],
                        scalar=typb[:, s * 3 + c:s * 3 + c + 1], in1=a_c,
                        op0=OP.mult, op1=OP.add)

            # ---------------- collective 2 (region min/max) ----------------
            nc.vector.tensor_reduce(out=rmm2[:, 0:1], in_=acc[:], axis=AX.X,
                                    op=OP.max)
            nc.vector.tensor_reduce(out=mtmp[:], in_=acc[:], axis=AX.X,
                                    op=OP.min)
            nc.vector.tensor_scalar(out=rmm2[:, 1:2], in0=mtmp[:], scalar1=-1.0,
                                    scalar2=None, op0=OP.mult)
            p_t2 = p_tt[:, 128:256]
            nc.tensor.transpose(p_t2, rmm2[:], ident[:])
            nc.vector.memset(rpad[:], -3.0e38)
            nc.vector.tensor_reduce(out=rpad[0:2, 0:1], in_=p_t2, axis=AX.X,
                                    op=OP.max)
            nc.sync.dma_start(out=cin2[:], in_=rpad[:])
            if NO_COLL:
                nc.gpsimd.dma_start(
                    out=rsb[:], in_=cin2[0:2, 0:1].rearrange("p o -> o p"))
            else:
                nc.gpsimd.collective_compute(
                    "AllReduce", OP.max, replica_groups=[list(range(N_CORES))],
                    ins=[cin2.opt()], outs=[cout2.opt()])
                nc.gpsimd.dma_start(
                    out=rsb[:], in_=cout2[0:2, 0:1].rearrange("p o -> o p"))

            nc.vector.tensor_tensor(out=dd[:], in0=rsb[:, 0:1], in1=rsb[:, 1:2],
                                    op=OP.add)
            nc.vector.reciprocal(rinvd[:], dd[:])
            nc.vector.tensor_copy(rrow[:, 0:1], rinvd[:])
            nc.vector.tensor_tensor(out=rrow[:, 1:2], in0=rsb[:, 1:2],
                                    in1=rinvd[:], op=OP.mult)
            p_b2 = p_misc[:, 10:12]
            nc.tensor.matmul(p_b2, ones_r[:], rrow[:], start=True, stop=True)
            nc.vector.tensor_copy(rcp[:], p_b2)

            # ---------------- composite + SSE ----------------
            nc.vector.tensor_scalar(out=law[:], in0=law[:], scalar1=rcp[:, 1:2],
                                    scalar2=None, op0=OP.add)
            nc.vector.scalar_tensor_tensor(out=law[:], in0=acc[:],
                                           scalar=rcp[:, 0:1], in1=law[:],
                                           op0=OP.mult, op1=OP.add)
            nc.vector.tensor_scalar(out=law[:], in0=law[:], scalar1=0.0,
                                    scalar2=1.0, op0=OP.max, op1=OP.min)
            nc.vector.tensor_tensor(out=law[:], in0=law[:], in1=gtn[:],
                                    op=OP.subtract)
            nc.vector.scalar_tensor_tensor(out=law[:], in0=law[:],
                                           scalar=1.0, in1=law[:], op0=OP.mult,
                                           op1=OP.mult, accum_out=ssecol[:])
            p_s = p_misc[0:1, 12:13]
            nc.tensor.matmul(p_s, ones_c[:], ssecol[:], start=True,
                             stop=True)
            nc.vector.tensor_copy(sse_sb[:], p_s)
            nc.sync.dma_start(out=sse.ap(), in_=sse_sb[:])

            # ---------------- debug outputs ----------------
            nc.sync.dma_start(out=dbg.ap()[:, 0:10], in_=rowb[:, 0:10])
            nc.sync.dma_start(out=dbg.ap()[:, 10:12], in_=rsb[:])
            nc.sync.dma_start(out=dbg.ap()[:, 12:14], in_=gsb[:])
            nc.sync.dma_start(out=dbg2.ap()[:, 0:126], in_=drow[:])
            nc.sync.dma_start(out=dbg2.ap()[:, 126:168], in_=tyrow)

    nc.finalize()
    return nc


_NC = None


def _get_nc():
    global _NC
    if _NC is None:
        _NC = build()
    return _NC


def run(gt_full, pred_full, trace=False):
    """Run the SPMD kernel on the full (8, ...) inputs. Returns
    (loss, BassKernelResults)."""
    nc = _get_nc()
    in_maps = [
        {"GT": np.ascontiguousarray(gt_full[i]),
         "Pred": np.ascontiguousarray(pred_full[i])}
        for i in range(N_CORES)
    ]
    res = run_bass_kernel_spmd(nc, in_maps, core_ids=list(range(N_CORES)),
                               trace=trace)
    total = sum(float(res.results[c]["sse"][0, 0]) for c in range(N_CORES))
    loss = np.float32(total / NPIX)
    return loss, res


def kernel(GT, Pred):
    gt_full = np.asarray(GT, dtype=np.float32)
    pred_full = np.asarray(Pred, dtype=np.float32)
    loss, _ = run(gt_full, pred_full, trace=False)
    return loss


if __name__ == "__main__":
    rng = np.random.default_rng(0)
    gt = rng.random((8, 4, H, W), dtype=np.float32)
    pr = rng.random((8, 6, SQE, 4, H, W), dtype=np.float32)
    print("loss:", kernel(gt, pr))
